# revision 49
# baseline (speedup 1.0000x reference)
"""Trainium2 Bass kernel for nn_GroupedLinear (16-group LayerNorm+Linear).

Problem: x [1024, 8, 64, 64] fp32; per group g (16 groups of 64 channels):
  X_g = contiguous 2M-element chunk g viewed row-major as [32768, 64]
  Y_g = LayerNorm(X_g) * gamma_g + beta_g  @ W_g^T + b_g      [32768, 64]
  out chunk g = Y_g^T  (contiguous [64, 32768] block of the output)

Sharding: expert-parallel, 2 groups per core across 8 cores. Each core's
input (2 x 8MB) and output (16.8MB) are disjoint contiguous DRAM blocks;
no collectives.

Per-core dataflow (1024-row macro-tiles; first/last macro split into
512-row halves to shorten pipeline fill and drain; steady-state period
~3.37us/tile, vector-engine bound):
  sync-q DMA in fp32 (one contiguous 256KB read per group; row r=8p+b)
  -> 8x bn_stats (V, groups interleaved) -> rstd via 2x ACT
     Abs_reciprocal_sqrt written as adjacent bf16 duplicate pairs
  -> (x-mu) on DVE (broadcast mu AP), cast bf16
  -> (*rstd) on DVE in 2x packed mode (bf16 pair trick); gpsimd unused
     so the DVE/Pool shared SBUF port never contends
  -> 8x PE transpose of [128, (2g,64ch)] stripes -> PSUM (b,q) col order
  -> ACT copy PSUM->SBUF bf16 with strided read = free row un-permute
  -> 2x matmul with block-diag [128,128] bf16 weights (gamma folded)
  -> ACT bias add (W@beta+b folded) + cast bf16 -> sync-q DMA out
Output DRAM tensor is bf16 (halves write traffic; tolerance 2e-2 rel);
host casts back to fp32.
"""

import sys

for _p in ("/opt/trn_rl_repo", "/opt/pypackages"):
    if _p not in sys.path:
        sys.path.insert(0, _p)

import numpy as np
import ml_dtypes

G_TOTAL = 16
N_CORES = 8
G_PER_CORE = G_TOTAL // N_CORES  # 2
IN_G = 64
OUT_G = 64
ROWS = 8 * 64 * 64  # 32768 rows per group
MACRO = 1024  # rows per macro-tile
NB = MACRO // 128  # 8 row-blocks per macro (partition p = rows NB*p+bb)
NMAC = ROWS // MACRO  # 32
EPS = 1e-6

_CACHE = {}


def _build_bass(rep=1):
    import concourse.bacc as bacc
    import concourse.bass as bass
    import concourse.tile as tile
    from concourse import mybir

    nc = bacc.Bacc(None, target_bir_lowering=False)

    x = nc.dram_tensor("x", [G_PER_CORE, ROWS, IN_G], mybir.dt.float32,
                       kind="ExternalInput")
    wb = nc.dram_tensor("wb", [128, 128], mybir.dt.bfloat16,
                        kind="ExternalInput")
    tb = nc.dram_tensor("tb", [128, 1], mybir.dt.float32,
                        kind="ExternalInput")
    ident = nc.dram_tensor("ident", [128, 128], mybir.dt.bfloat16,
                           kind="ExternalInput")
    out = nc.dram_tensor("out", [128, ROWS], mybir.dt.bfloat16,
                         kind="ExternalOutput")

    F = mybir.ActivationFunctionType
    A = mybir.AluOpType

    with tile.TileContext(nc, pool_alloc_mode="queue") as tc:
        with (
            tc.tile_pool(name="singles", bufs=1) as singles,
            tc.tile_pool(name="xload", bufs=4) as xload,
            tc.tile_pool(name="statp", bufs=4) as statp,
            tc.tile_pool(name="mvp", bufs=4) as mvp,
            tc.tile_pool(name="rstdp", bufs=4) as rstdp,
            tc.tile_pool(name="xnp", bufs=4) as xnp,
            tc.tile_pool(name="xtsp", bufs=4) as xtsp,
            tc.tile_pool(name="youtp", bufs=4) as youtp,
            tc.tile_pool(name="xtpp", bufs=3, space="PSUM") as xtpp,
            tc.tile_pool(name="ypp", bufs=2, space="PSUM") as ypp,
        ):
            sb_wb = singles.tile([128, 128], mybir.dt.bfloat16)
            sb_tb = singles.tile([128, 1], mybir.dt.float32)
            sb_id = singles.tile([128, 128], mybir.dt.bfloat16)
            sb_eps = singles.tile([128, 1], mybir.dt.float32)
            # singles on the scalar queue: keeps sync's queue free so the
            # first x-tile load issues immediately at startup
            nc.scalar.dma_start(out=sb_wb, in_=wb[:, :])
            nc.scalar.dma_start(out=sb_tb, in_=tb[:, :])
            nc.scalar.dma_start(out=sb_id, in_=ident[:, :])
            nc.vector.memset(sb_eps, EPS)

            def front_end(r0, nb, first=False):
                """load, stats, rstd, sub for rows [r0, r0+nb*128)."""
                rows = nb * 128
                x_t = xload.tile([128, G_PER_CORE, nb, IN_G],
                                 mybir.dt.float32)
                # contiguous per (partition, group): row r = nb*p + b.
                # One DMA per group: each is a single fully-contiguous
                # DRAM read. For the first tile the two group loads go to
                # different queues so they transfer in parallel (cuts the
                # pipeline-fill latency).
                for g in range(G_PER_CORE):
                    nc.sync.dma_start(
                        out=x_t[:, g, :, :],
                        in_=x[g, r0:r0 + rows, :].rearrange(
                            "(p b) c -> p b c", p=128),
                    )

                # ---- stats: one bn_stats per block, groups interleaved
                st = statp.tile([128, nb, 6], mybir.dt.float32)
                for bb in range(nb):
                    in3 = x_t[:, :, bb, :].rearrange("p g c -> p c g")
                    nc.vector.add_instruction(
                        mybir.InstBNStats(
                            name=nc.get_next_instruction_name(),
                            ins=[nc.vector.lower_ap(in3)],
                            outs=[nc.vector.lower_ap(st[:, bb, :])],
                        )
                    )
                # rstd = 1/sqrt((count*var)/64 + eps) written twice as
                # adjacent bf16 duplicate pairs [128, b, g, pair] so the
                # DVE mul can run in 2x packed mode
                rstd = rstdp.tile([128, nb, 2, 2], mybir.dt.bfloat16)
                for k in range(2):
                    nc.scalar.activation(out=rstd[:, :, :, k],
                                         in_=st[:, :, 2:6:3],
                                         func=F.Abs_reciprocal_sqrt,
                                         bias=sb_eps[:, 0:1],
                                         scale=1.0 / IN_G)

                # ---- (x - mu) on DVE, cast to bf16 (Pool must stay idle:
                # any sustained Pool op slows all concurrent DVE ops via
                # the shared SBUF port pair)
                xn = xnp.tile([128, nb, G_PER_CORE, IN_G],
                              mybir.dt.bfloat16)
                st_ap = st[:, :, :]
                xn_v = xn.rearrange("p b g c -> p g b c")
                mu_b = bass.AP(
                    tensor=st_ap.tensor, offset=st_ap.offset + 1,
                    ap=[st_ap.ap[0], [3, G_PER_CORE], [6, nb], [0, IN_G]],
                )
                nc.vector.tensor_sub(xn_v, x_t[:, :, :, :], mu_b)
                return r0, nb, xn, rstd

            def back_end(state):
                """mul, transpose, copy, matmul, bias, store."""
                r0, nb, xn, rstd = state
                rows = nb * 128
                # ---- * rstd on DVE in 2x packed mode (bf16 duplicate
                # pairs)
                r2_ap = rstd[:, :, :, :]
                for g in range(G_PER_CORE):
                    rstd_pair = bass.AP(
                        tensor=r2_ap.tensor, offset=r2_ap.offset + 2 * g,
                        ap=[r2_ap.ap[0], [4, nb], [0, IN_G // 2], [1, 2]],
                    )
                    nc.vector.tensor_mul(xn[:, :, g, :], xn[:, :, g, :],
                                         rstd_pair)

                # ---- transpose nb stripes -> PSUM [128, rows]
                xtp = xtpp.tile([128, rows], mybir.dt.bfloat16)
                for bb in range(nb):
                    nc.tensor.transpose(
                        out=xtp[:, bb * 128:(bb + 1) * 128],
                        in_=xn[:, bb, :, :].rearrange("p g c -> p (g c)"),
                        identity=sb_id,
                    )

                # ---- PSUM -> SBUF bf16 (ACT), un-permuting rows via a
                # strided read: xtp col (b,q) -> xts col r=nb*q+b
                xts = xtsp.tile([128, rows], mybir.dt.bfloat16)
                xtp_v = xtp.rearrange("p (b q) -> p q b", b=nb)
                nc.scalar.activation(
                    out=xts.rearrange("p (q b) -> p q b", b=nb),
                    in_=xtp_v, func=F.Copy)

                # ---- matmul: both groups per op via block-diag weights
                yp = ypp.tile([128, rows], mybir.dt.float32)
                nmm = max(1, rows // 512)
                w = rows // nmm
                for k in range(nmm):
                    nc.tensor.matmul(yp[:, k * w:(k + 1) * w],
                                     lhsT=sb_wb,
                                     rhs=xts[:, k * w:(k + 1) * w],
                                     start=True, stop=True)

                # ---- bias add + cast bf16 (rows already in natural order)
                y_t = youtp.tile([128, rows], mybir.dt.bfloat16)
                nc.scalar.activation(out=y_t, in_=yp, func=F.Identity,
                                     bias=sb_tb[:, 0:1], scale=1.0)

                nc.sync.dma_start(out=out[:, r0:r0 + rows], in_=y_t)

            # worklist: edge macros split into quarter/half tiles to
            # shorten pipeline fill and drain; full 1024-row tiles between
            QB = NB // 4
            HB = NB // 2
            work = [(0, QB), (QB * 128, QB), (2 * QB * 128, HB)]
            work += [(m * MACRO, NB) for m in range(1, NMAC - 1)]
            last = (NMAC - 1) * MACRO
            work += [(last, HB), (last + HB * 128, QB),
                     (last + 3 * QB * 128, QB)]

            # software-pipelined emission: back-end of tile t-1 is emitted
            # after front-end of tile t
            prev = None
            for rr in range(rep):
                for ti, (r0, nb) in enumerate(work):
                    state = front_end(r0, nb, first=(ti == 0 and rr == 0))
                    if prev is not None:
                        back_end(prev)
                    prev = state
            back_end(prev)

    nc.finalize()
    return nc


def _get_nc(rep=1):
    key = ("nc", rep)
    if key not in _CACHE:
        _CACHE[key] = _build_bass(rep)
    return _CACHE[key]


def _make_in_maps(x, ln_gamma, ln_beta, W, b):
    bf16 = ml_dtypes.bfloat16
    xg = np.ascontiguousarray(x.reshape(G_TOTAL, ROWS, IN_G))
    in_maps = []
    for c in range(N_CORES):
        gs = [G_PER_CORE * c + g for g in range(G_PER_CORE)]
        wbc = np.zeros((128, 128), np.float32)
        tbc = np.zeros((128, 1), np.float32)
        for g_local, g in enumerate(gs):
            Wp = W[g] * ln_gamma[g][None, :]  # [out, in] gamma folded
            lo = g_local * 64
            wbc[lo:lo + 64, lo:lo + 64] = Wp.T  # lhsT[k=in, m=out]
            tbc[lo:lo + 64, 0] = W[g] @ ln_beta[g] + b[g]
        in_maps.append({
            "x": np.ascontiguousarray(xg[gs[0]:gs[-1] + 1]),
            "wb": wbc.astype(bf16),
            "tb": tbc,
            "ident": np.eye(128, dtype=np.float32).astype(bf16),
        })
    return in_maps


def _run(in_maps, trace=False):
    from concourse.bass_utils import run_bass_kernel_spmd
    nc = _get_nc()
    return run_bass_kernel_spmd(nc, in_maps, list(range(N_CORES)),
                                trace=trace)


def bench(in_maps, rep, iters=12):
    """Time repeated on-device executions of the rep-times-unrolled kernel.

    Returns list of per-call wall times (s). Per-iteration kernel time is
    estimated by the caller from the difference between two rep values.
    """
    import time
    import jax
    import jax.numpy as jnp
    import numpy as np_
    from jax.sharding import Mesh, PartitionSpec
    from jax.experimental.shard_map import shard_map
    from concourse import bass2jax
    from concourse import mybir

    bass2jax.install_neuronx_cc_hook()
    nc = _get_nc(rep)

    partition_name = (nc.partition_id_tensor.name
                      if nc.partition_id_tensor else None)
    in_names, out_names, out_avals = [], [], []
    zero_shapes = []
    for alloc in nc.m.functions[0].allocations:
        if not isinstance(alloc, mybir.MemoryLocationSet):
            continue
        name = alloc.memorylocations[0].name
        if alloc.kind == "ExternalInput":
            if name != partition_name:
                in_names.append(name)
        elif alloc.kind == "ExternalOutput":
            out_names.append(name)
            shape = tuple(alloc.tensor_shape)
            dtype = mybir.dt.np(alloc.dtype)
            out_avals.append(jax.core.ShapedArray(shape, dtype))
            zero_shapes.append((shape, dtype))
    n_params = len(in_names)
    all_names = list(in_names) + out_names
    if partition_name is not None:
        all_names.append(partition_name)

    def _body(*args):
        operands = list(args)
        if partition_name is not None:
            operands.append(bass2jax.partition_id_tensor())
        outs = bass2jax._bass_exec_p.bind(
            *operands,
            out_avals=tuple(out_avals),
            in_names=tuple(all_names),
            out_names=tuple(out_names),
            lowering_input_output_aliases=(),
            sim_require_finite=True,
            sim_require_nnan=True,
            nc=nc,
        )
        return tuple(outs)

    n_cores = len(in_maps)
    devices = jax.devices()[:n_cores]
    mesh = Mesh(np_.asarray(devices), ("core",))
    nzero = len(zero_shapes)
    in_specs = (PartitionSpec("core"),) * (n_params + nzero)
    out_specs = (PartitionSpec("core"),) * len(out_names)
    donate = tuple(range(n_params, n_params + nzero))
    sharded = jax.jit(
        shard_map(_body, mesh=mesh, in_specs=in_specs,
                  out_specs=out_specs, check_rep=False),
        donate_argnums=donate, keep_unused=True)

    concat_in = [
        jax.device_put(
            np_.concatenate([np_.asarray(in_maps[c][name])
                             for c in range(n_cores)], axis=0))
        for name in in_names
    ]

    def make_zeros():
        return [
            jnp.zeros((shape[0] * n_cores,) + tuple(shape[1:]), dtype)
            for shape, dtype in zero_shapes
        ]

    times = []
    for i in range(iters):
        zs = [jax.device_put(z) for z in make_zeros()]
        for z in zs:
            z.block_until_ready()
        t0 = time.perf_counter()
        outs = sharded(*concat_in, *zs)
        for o in outs:
            o.block_until_ready()
        times.append(time.perf_counter() - t0)
    return times


def kernel(x, ln_gamma, ln_beta, W, b):
    x = np.asarray(x, np.float32)
    ln_gamma = np.asarray(ln_gamma, np.float32)
    ln_beta = np.asarray(ln_beta, np.float32)
    W = np.asarray(W, np.float32)
    b = np.asarray(b, np.float32)

    in_maps = _make_in_maps(x, ln_gamma, ln_beta, W, b)
    res = _run(in_maps, trace=False)
    outs = [np.asarray(r["out"]).astype(np.float32) for r in res.results]
    full = np.concatenate(outs, axis=0)  # [1024, 32768]
    return full.reshape(1024, 8, 64, 64)



# revision 50
# speedup vs baseline: 1.0179x; 1.0179x over previous
"""Trainium2 Bass kernel for nn_GroupedLinear (16-group LayerNorm+Linear).

Problem: x [1024, 8, 64, 64] fp32; per group g (16 groups of 64 channels):
  X_g = contiguous 2M-element chunk g viewed row-major as [32768, 64]
  Y_g = LayerNorm(X_g) * gamma_g + beta_g  @ W_g^T + b_g      [32768, 64]
  out chunk g = Y_g^T  (contiguous [64, 32768] block of the output)

Sharding: expert-parallel, 2 groups per core across 8 cores. Each core's
input (2 x 8MB) and output (16.8MB) are disjoint contiguous DRAM blocks;
no collectives.

Per-core dataflow (1024-row macro-tiles; first/last macro split into
512-row halves to shorten pipeline fill and drain; steady-state period
~3.37us/tile, vector-engine bound):
  sync-q DMA in fp32 (one contiguous 256KB read per group; row r=8p+b)
  -> 8x bn_stats (V, groups interleaved) -> rstd via 2x ACT
     Abs_reciprocal_sqrt written as adjacent bf16 duplicate pairs
  -> (x-mu) on DVE (broadcast mu AP), cast bf16
  -> (*rstd) on DVE in 2x packed mode (bf16 pair trick); gpsimd unused
     so the DVE/Pool shared SBUF port never contends
  -> 8x PE transpose of [128, (2g,64ch)] stripes -> PSUM (b,q) col order
  -> ACT copy PSUM->SBUF bf16 with strided read = free row un-permute
  -> 2x matmul with block-diag [128,128] bf16 weights (gamma folded)
  -> ACT bias add (W@beta+b folded) + cast bf16 -> sync-q DMA out
Output DRAM tensor is bf16 (halves write traffic; tolerance 2e-2 rel);
host casts back to fp32.
"""

import sys

for _p in ("/opt/trn_rl_repo", "/opt/pypackages"):
    if _p not in sys.path:
        sys.path.insert(0, _p)

import numpy as np
import ml_dtypes

G_TOTAL = 16
N_CORES = 8
G_PER_CORE = G_TOTAL // N_CORES  # 2
IN_G = 64
OUT_G = 64
ROWS = 8 * 64 * 64  # 32768 rows per group
MACRO = 1024  # rows per macro-tile
NB = MACRO // 128  # 8 row-blocks per macro (partition p = rows NB*p+bb)
NMAC = ROWS // MACRO  # 32
EPS = 1e-6

_CACHE = {}


def _build_bass(rep=1):
    import concourse.bacc as bacc
    import concourse.bass as bass
    import concourse.tile as tile
    from concourse import mybir

    nc = bacc.Bacc(None, target_bir_lowering=False)

    x = nc.dram_tensor("x", [G_PER_CORE, ROWS, IN_G], mybir.dt.float32,
                       kind="ExternalInput")
    wb = nc.dram_tensor("wb", [128, 128], mybir.dt.bfloat16,
                        kind="ExternalInput")
    tb = nc.dram_tensor("tb", [128, 1], mybir.dt.float32,
                        kind="ExternalInput")
    ident = nc.dram_tensor("ident", [128, 128], mybir.dt.bfloat16,
                           kind="ExternalInput")
    out = nc.dram_tensor("out", [128, ROWS], mybir.dt.bfloat16,
                         kind="ExternalOutput")

    F = mybir.ActivationFunctionType
    A = mybir.AluOpType

    with tile.TileContext(nc, pool_alloc_mode="queue") as tc:
        with (
            tc.tile_pool(name="singles", bufs=1) as singles,
            tc.tile_pool(name="xload", bufs=4) as xload,
            tc.tile_pool(name="statp", bufs=4) as statp,
            tc.tile_pool(name="mvp", bufs=4) as mvp,
            tc.tile_pool(name="rstdp", bufs=4) as rstdp,
            tc.tile_pool(name="xnp", bufs=4) as xnp,
            tc.tile_pool(name="xtsp", bufs=4) as xtsp,
            tc.tile_pool(name="youtp", bufs=4) as youtp,
            tc.tile_pool(name="xtpp", bufs=3, space="PSUM") as xtpp,
            tc.tile_pool(name="ypp", bufs=2, space="PSUM") as ypp,
        ):
            sb_wb = singles.tile([128, 128], mybir.dt.bfloat16)
            sb_tb = singles.tile([128, 1], mybir.dt.float32)
            sb_id = singles.tile([128, 128], mybir.dt.bfloat16)
            sb_eps = singles.tile([128, 1], mybir.dt.float32)
            # singles on the scalar queue: keeps sync's queue free so the
            # first x-tile load issues immediately at startup
            nc.scalar.dma_start(out=sb_wb, in_=wb[:, :])
            nc.scalar.dma_start(out=sb_tb, in_=tb[:, :])
            nc.scalar.dma_start(out=sb_id, in_=ident[:, :])
            nc.vector.memset(sb_eps, EPS)

            def front_end(r0, nb, first=False):
                """load, stats, rstd, sub for rows [r0, r0+nb*128)."""
                rows = nb * 128
                x_t = xload.tile([128, G_PER_CORE, nb, IN_G],
                                 mybir.dt.float32)
                # contiguous per (partition, group): row r = nb*p + b.
                # One DMA per group: each is a single fully-contiguous
                # DRAM read. For the first tile the two group loads go to
                # different queues so they transfer in parallel (cuts the
                # pipeline-fill latency).
                for g in range(G_PER_CORE):
                    nc.sync.dma_start(
                        out=x_t[:, g, :, :],
                        in_=x[g, r0:r0 + rows, :].rearrange(
                            "(p b) c -> p b c", p=128),
                    )

                # ---- stats: one bn_stats per block, groups interleaved
                st = statp.tile([128, nb, 6], mybir.dt.float32)
                for bb in range(nb):
                    in3 = x_t[:, :, bb, :].rearrange("p g c -> p c g")
                    nc.vector.add_instruction(
                        mybir.InstBNStats(
                            name=nc.get_next_instruction_name(),
                            ins=[nc.vector.lower_ap(in3)],
                            outs=[nc.vector.lower_ap(st[:, bb, :])],
                        )
                    )
                # rstd = 1/sqrt((count*var)/64 + eps) written twice as
                # adjacent bf16 duplicate pairs [128, b, g, pair] so the
                # DVE mul can run in 2x packed mode
                rstd = rstdp.tile([128, nb, 2, 2], mybir.dt.bfloat16)
                for k in range(2):
                    nc.scalar.activation(out=rstd[:, :, :, k],
                                         in_=st[:, :, 2:6:3],
                                         func=F.Abs_reciprocal_sqrt,
                                         bias=sb_eps[:, 0:1],
                                         scale=1.0 / IN_G)

                # ---- (x - mu) on DVE, cast to bf16 (Pool must stay idle:
                # any sustained Pool op slows all concurrent DVE ops via
                # the shared SBUF port pair)
                xn = xnp.tile([128, nb, G_PER_CORE, IN_G],
                              mybir.dt.bfloat16)
                st_ap = st[:, :, :]
                xn_v = xn.rearrange("p b g c -> p g b c")
                mu_b = bass.AP(
                    tensor=st_ap.tensor, offset=st_ap.offset + 1,
                    ap=[st_ap.ap[0], [3, G_PER_CORE], [6, nb], [0, IN_G]],
                )
                nc.vector.tensor_sub(xn_v, x_t[:, :, :, :], mu_b)
                return r0, nb, xn, rstd

            def back_end(state):
                """mul, transpose, copy, matmul, bias, store."""
                r0, nb, xn, rstd = state
                rows = nb * 128
                # ---- * rstd on DVE in 2x packed mode (bf16 duplicate
                # pairs)
                r2_ap = rstd[:, :, :, :]
                for g in range(G_PER_CORE):
                    rstd_pair = bass.AP(
                        tensor=r2_ap.tensor, offset=r2_ap.offset + 2 * g,
                        ap=[r2_ap.ap[0], [4, nb], [0, IN_G // 2], [1, 2]],
                    )
                    nc.vector.tensor_mul(xn[:, :, g, :], xn[:, :, g, :],
                                         rstd_pair)

                # ---- transpose nb stripes -> PSUM [128, rows]
                xtp = xtpp.tile([128, rows], mybir.dt.bfloat16)
                for bb in range(nb):
                    nc.tensor.transpose(
                        out=xtp[:, bb * 128:(bb + 1) * 128],
                        in_=xn[:, bb, :, :].rearrange("p g c -> p (g c)"),
                        identity=sb_id,
                    )

                # ---- PSUM -> SBUF bf16 (ACT), un-permuting rows via a
                # strided read: xtp col (b,q) -> xts col r=nb*q+b
                xts = xtsp.tile([128, rows], mybir.dt.bfloat16)
                xtp_v = xtp.rearrange("p (b q) -> p q b", b=nb)
                nc.scalar.activation(
                    out=xts.rearrange("p (q b) -> p q b", b=nb),
                    in_=xtp_v, func=F.Copy)

                # ---- matmul: both groups per op via block-diag weights
                yp = ypp.tile([128, rows], mybir.dt.float32)
                for k in range(rows // 512):
                    nc.tensor.matmul(yp[:, k * 512:(k + 1) * 512],
                                     lhsT=sb_wb,
                                     rhs=xts[:, k * 512:(k + 1) * 512],
                                     start=True, stop=True)

                # ---- bias add + cast bf16 (rows already in natural order)
                y_t = youtp.tile([128, rows], mybir.dt.bfloat16)
                nc.scalar.activation(out=y_t, in_=yp, func=F.Identity,
                                     bias=sb_tb[:, 0:1], scale=1.0)

                nc.sync.dma_start(out=out[:, r0:r0 + rows], in_=y_t)

            # worklist: first and last macro split into 512-row halves to
            # shorten pipeline fill and drain; full 1024-row tiles between
            HB = NB // 2
            work = [(0, HB), (HB * 128, HB)]
            work += [(m * MACRO, NB) for m in range(1, NMAC - 1)]
            last = (NMAC - 1) * MACRO
            work += [(last, HB), (last + HB * 128, HB)]

            # software-pipelined emission: back-end of tile t-1 is emitted
            # after front-end of tile t
            prev = None
            for rr in range(rep):
                for ti, (r0, nb) in enumerate(work):
                    state = front_end(r0, nb, first=(ti == 0 and rr == 0))
                    if prev is not None:
                        back_end(prev)
                    prev = state
            back_end(prev)

    nc.finalize()
    return nc


def _get_nc(rep=1):
    key = ("nc", rep)
    if key not in _CACHE:
        _CACHE[key] = _build_bass(rep)
    return _CACHE[key]


def _make_in_maps(x, ln_gamma, ln_beta, W, b):
    bf16 = ml_dtypes.bfloat16
    xg = np.ascontiguousarray(x.reshape(G_TOTAL, ROWS, IN_G))
    in_maps = []
    for c in range(N_CORES):
        gs = [G_PER_CORE * c + g for g in range(G_PER_CORE)]
        wbc = np.zeros((128, 128), np.float32)
        tbc = np.zeros((128, 1), np.float32)
        for g_local, g in enumerate(gs):
            Wp = W[g] * ln_gamma[g][None, :]  # [out, in] gamma folded
            lo = g_local * 64
            wbc[lo:lo + 64, lo:lo + 64] = Wp.T  # lhsT[k=in, m=out]
            tbc[lo:lo + 64, 0] = W[g] @ ln_beta[g] + b[g]
        in_maps.append({
            "x": np.ascontiguousarray(xg[gs[0]:gs[-1] + 1]),
            "wb": wbc.astype(bf16),
            "tb": tbc,
            "ident": np.eye(128, dtype=np.float32).astype(bf16),
        })
    return in_maps


def _run(in_maps, trace=False):
    from concourse.bass_utils import run_bass_kernel_spmd
    nc = _get_nc()
    return run_bass_kernel_spmd(nc, in_maps, list(range(N_CORES)),
                                trace=trace)


def bench(in_maps, rep, iters=12):
    """Time repeated on-device executions of the rep-times-unrolled kernel.

    Returns list of per-call wall times (s). Per-iteration kernel time is
    estimated by the caller from the difference between two rep values.
    """
    import time
    import jax
    import jax.numpy as jnp
    import numpy as np_
    from jax.sharding import Mesh, PartitionSpec
    from jax.experimental.shard_map import shard_map
    from concourse import bass2jax
    from concourse import mybir

    bass2jax.install_neuronx_cc_hook()
    nc = _get_nc(rep)

    partition_name = (nc.partition_id_tensor.name
                      if nc.partition_id_tensor else None)
    in_names, out_names, out_avals = [], [], []
    zero_shapes = []
    for alloc in nc.m.functions[0].allocations:
        if not isinstance(alloc, mybir.MemoryLocationSet):
            continue
        name = alloc.memorylocations[0].name
        if alloc.kind == "ExternalInput":
            if name != partition_name:
                in_names.append(name)
        elif alloc.kind == "ExternalOutput":
            out_names.append(name)
            shape = tuple(alloc.tensor_shape)
            dtype = mybir.dt.np(alloc.dtype)
            out_avals.append(jax.core.ShapedArray(shape, dtype))
            zero_shapes.append((shape, dtype))
    n_params = len(in_names)
    all_names = list(in_names) + out_names
    if partition_name is not None:
        all_names.append(partition_name)

    def _body(*args):
        operands = list(args)
        if partition_name is not None:
            operands.append(bass2jax.partition_id_tensor())
        outs = bass2jax._bass_exec_p.bind(
            *operands,
            out_avals=tuple(out_avals),
            in_names=tuple(all_names),
            out_names=tuple(out_names),
            lowering_input_output_aliases=(),
            sim_require_finite=True,
            sim_require_nnan=True,
            nc=nc,
        )
        return tuple(outs)

    n_cores = len(in_maps)
    devices = jax.devices()[:n_cores]
    mesh = Mesh(np_.asarray(devices), ("core",))
    nzero = len(zero_shapes)
    in_specs = (PartitionSpec("core"),) * (n_params + nzero)
    out_specs = (PartitionSpec("core"),) * len(out_names)
    donate = tuple(range(n_params, n_params + nzero))
    sharded = jax.jit(
        shard_map(_body, mesh=mesh, in_specs=in_specs,
                  out_specs=out_specs, check_rep=False),
        donate_argnums=donate, keep_unused=True)

    concat_in = [
        jax.device_put(
            np_.concatenate([np_.asarray(in_maps[c][name])
                             for c in range(n_cores)], axis=0))
        for name in in_names
    ]

    def make_zeros():
        return [
            jnp.zeros((shape[0] * n_cores,) + tuple(shape[1:]), dtype)
            for shape, dtype in zero_shapes
        ]

    times = []
    for i in range(iters):
        zs = [jax.device_put(z) for z in make_zeros()]
        for z in zs:
            z.block_until_ready()
        t0 = time.perf_counter()
        outs = sharded(*concat_in, *zs)
        for o in outs:
            o.block_until_ready()
        times.append(time.perf_counter() - t0)
    return times


def kernel(x, ln_gamma, ln_beta, W, b):
    x = np.asarray(x, np.float32)
    ln_gamma = np.asarray(ln_gamma, np.float32)
    ln_beta = np.asarray(ln_beta, np.float32)
    W = np.asarray(W, np.float32)
    b = np.asarray(b, np.float32)

    in_maps = _make_in_maps(x, ln_gamma, ln_beta, W, b)
    res = _run(in_maps, trace=False)
    outs = [np.asarray(r["out"]).astype(np.float32) for r in res.results]
    full = np.concatenate(outs, axis=0)  # [1024, 32768]
    return full.reshape(1024, 8, 64, 64)

